# revision 23
# baseline (speedup 1.0000x reference)
"""Trainium2 Bass kernel for DeepAngAEVComputer (angular AEV: per-triplet MLP
with weighted per-atom scatter-add).

Contract: kernel(**inputs) takes the FULL unsharded inputs (B=8 molecules) and
returns the FULL [8, 32, 256] output.

Sharding: by ATOM, load-balanced.  Only triplets (i;j,k) with both R_ij and
R_ik inside the 3.5 cutoff contribute (w=0 otherwise); for these inputs that
is ~3.4k of 127k triplets.  The host enumerates surviving triplets per atom,
bin-packs the 256 (molecule, atom) pairs onto 8 cores x 32 output slots
(whole atoms, so the final normalization stays on-device), and pads each
core to T tokens (T=512 default; compile-on-demand ladder up to 16384 for
inputs with more surviving triplets).  The device kernel computes the 9
triplet features, the residual MLP, the cutoff weights and the per-slot
weighted scatter-add + normalization.  Host-side work is only selection /
layout; all reference FLOPs stay on device.

Per-core layout (per chunk of 128*CC tokens; CC=4 for T=512):
  token (a, l): strip a in [0,4), l in [0, 32*CC).  Feature stage holds
  token-major maps FB[32a + l%32, slot, l//32].  A 32x32 block transpose
  yields xfm[32a+f, l] = feature-major activations.  The MLP packs two
  strips per 128-partition matmul (block-stacked / block-diagonal stationary
  weights), all-tanh on the scalar engine with per-partition biases.  The
  last layer runs token-major (128-token blocks on PSUM partitions via
  stationary xb3 slices) so the weighted scatter-add becomes accumulating
  [128,32]x[128,256] matmuls into a persistent [32,256] PSUM tile.
  All sqrt/rsqrt run on the vector engine (bit-trick + Newton-Raphson), so
  the scalar engine only ever needs sin + tanh -> a single activation table
  load (silu_and_others) for the whole kernel.
"""

import os
from contextlib import ExitStack

import ml_dtypes
import numpy as np

import concourse.bass as bass
import concourse.tile as tile
from concourse import bacc
from concourse import mybir
from concourse.bass_utils import run_bass_kernel_spmd

F32 = mybir.dt.float32
F16 = mybir.dt.float16
BF16 = mybir.dt.bfloat16
U32 = mybir.dt.uint32
ALU = mybir.AluOpType
ACTF = mybir.ActivationFunctionType

CUTOFF = 3.5
EPS = 1e-7
CLIP_MIN = 1e-10
PI = float(np.pi)
MAGIC = 0x5F3759DF  # Quake rsqrt seed

NCORE = 8
NA = 32

# ladder of (CC, NCHUNK): T = 128*CC*NCHUNK tokens per core
LADDER = [(4, 1), (8, 1), (16, 1), (16, 2), (16, 4), (16, 8)]

_JI, _KI = np.triu_indices(NA, k=1)

# wpack16 column offsets (fp16 matmul operands + b6 broadcast)
_W16 = {"w0sA": 0, "w0sB": 128, "w1d": 256, "w2d": 384, "w3d": 512,
        "w4d": 640, "w5p0": 768, "w5p1": 896, "w6": 1024, "b6bc": 1280}
W16_COLS = 1536
# wf32 columns: per-partition ACT biases
_BIAS = {"b0": 0, "b1": 1, "b2": 2, "b3": 3, "b4": 4, "b5": 5}


# --------------------------------------------------------------------------
# AP helpers
# --------------------------------------------------------------------------

def _ap(base, dims):
    """AP with base's tensor/offset/partition dim and custom free dims."""
    return bass.AP(tensor=base.tensor, offset=base.offset,
                   ap=[list(base.ap[0])] + [list(d) for d in dims])


def slot(fb, i, n=1, step=1, cc=4):
    """[128, n, cc] view of slots i, i+step, ... of FB-like tile [128, S, cc]."""
    base = fb[:, i, :]
    return _ap(base, [[step * cc, n], [1, cc]])


def slot_bc(fb, i, n, cc):
    """slot i broadcast n times along the slot axis."""
    base = fb[:, i, :]
    return _ap(base, [[0, n], [1, cc]])


def slot_T(fb, i, n, cc):
    """[128, cc, n] reordered view (slots innermost) for tensor_reduce."""
    base = fb[:, i, :]
    return _ap(base, [[1, cc], [cc, n]])


# --------------------------------------------------------------------------
# device kernel
# --------------------------------------------------------------------------

def build_kernel(ctx, tc, out_ap, ins, CC, NCHUNK):
    nc = tc.nc
    TC = 128 * CC            # tokens per chunk
    SL = 32 * CC             # tokens per strip per chunk
    STAGE = int(os.environ.get("AEV_STAGE", "0"))  # debug bisect: 0=full

    def early_exit(src_ap, ncols):
        outs = consts.tile([32, 256], F32, tag="outs", name="outs")
        nc.vector.memset(outs[:], 0.0)
        nc.vector.tensor_copy(outs[:, 0:ncols], src_ap)
        nc.sync.dma_start(out_ap[:], outs[:])

    consts = ctx.enter_context(tc.tile_pool(name="consts", bufs=1))
    fbp = ctx.enter_context(tc.tile_pool(name="fbp", bufs=min(2, NCHUNK)))
    actp = ctx.enter_context(tc.tile_pool(name="actp", bufs=6))
    smal = ctx.enter_context(tc.tile_pool(name="smal", bufs=2))
    psp = ctx.enter_context(
        tc.tile_pool(name="psp", bufs=3 if CC <= 8 else 2, space="PSUM"))
    ps6p = ctx.enter_context(tc.tile_pool(name="ps6p", bufs=2, space="PSUM"))
    gap = ctx.enter_context(tc.tile_pool(name="gap", bufs=1, space="PSUM"))

    # ---- constants / inputs ----
    wp = consts.tile([128, W16_COLS], F16, tag="wp", name="wp")
    nc.sync.dma_start(wp[:], ins["wpack16"][:])
    wb = consts.tile([128, 6], F32, tag="wb", name="wb")
    nc.sync.dma_start(wb[:], ins["wf32"][:])
    oh = consts.tile([128, NCHUNK * CC * 32], BF16, tag="oh", name="oh")
    nc.sync.dma_start(oh[:], ins["oh"][:])
    oh_v = oh[:].rearrange("p (n c u) -> p n c u", n=NCHUNK, c=CC)
    magic = consts.tile([128, 1], U32, tag="magic", name="magic")
    nc.vector.memset(magic[:], MAGIC)
    halfpi = consts.tile([128, 1], F32, tag="halfpi", name="halfpi")
    nc.vector.memset(halfpi[:], PI / 2)

    geom_d = ins["geom"][:].rearrange("p (n q c) -> p n q c", n=NCHUNK, q=6)

    def W(nm, n=128):
        c = _W16[nm]
        return wp[:, c:c + n]

    def B(nm):
        return wb[:, _BIAS[nm]:_BIAS[nm] + 1]

    ga = gap.tile([32, 256], F32, tag="ga", name="ga")

    V = nc.vector

    for ch in range(NCHUNK):
        FB = fbp.tile([128, 32, CC], F32, tag="FB", name="FB")
        FB2 = fbp.tile([128, 16, CC], F32, tag="FB2", name="FB2")
        fb9 = fbp.tile([128, CC, 32], F32, tag="fb9", name="fb9")
        xfm = fbp.tile([128, CC, 32], F16, tag="xfm", name="xfm")
        xb3 = fbp.tile([128, 4 * SL], F16, tag="xb3", name="xb3")
        ab = fbp.tile([128, CC, 32], F16, tag="ab", name="ab")

        nc.sync.dma_start(FB[:, 0:6, :], geom_d[:, ch, :, :])
        V.memset(fb9[:], 0.0)
        if STAGE == 5:
            return early_exit(FB[0:32, 0:6, :], 6 * CC)

        def S(i, n=1, step=1):
            return slot(FB, i, n, step, CC)

        def S2(i, n=1, step=1):
            return slot(FB2, i, n, step, CC)

        def TT(out, a, b, op):
            V.tensor_tensor(out=out, in0=a, in1=b, op=op)

        # ---- features (slots: 0 rij, 1 rik, 2 rjk, 3 zi, 4 zj, 5 zk) ----
        TT(S(6, 3), S(0, 3), S(0, 3), ALU.mult)          # sq_ij/ik/jk
        TT(S(9, 2), slot_bc(FB, 0, 2, CC), S(1, 2), ALU.mult)  # p_ijik,p_ijjk
        TT(S(11), S(1), S(2), ALU.mult)                  # p_ikjk
        V.tensor_scalar(out=S(12, 3), in0=S(9, 3), scalar1=2.0,
                        scalar2=CLIP_MIN, op0=ALU.mult, op1=ALU.max)
        V.reciprocal(out=S(12, 3), in_=S(12, 3))         # 1/den_i/j/k
        TT(S(15, 2), slot_bc(FB, 6, 2, CC), S(7, 2), ALU.add)
        TT(S(17), S(7), S(8), ALU.add)
        TT(S(15, 3), S(15, 3), S(8, 3, step=-1), ALU.subtract)  # numerators
        TT(S(18, 3), S(15, 3), S(12, 3), ALU.mult)       # c_i, c_j, c_k
        V.tensor_reduce(out=S(21), in_=slot_T(FB, 0, 3, CC), axis=mybir.AxisListType.X,
                        op=ALU.add)                      # g0
        V.tensor_reduce(out=S(22), in_=slot_T(FB, 9, 3, CC), axis=mybir.AxisListType.X,
                        op=ALU.add)                      # g1
        TT(S(23), S(9), S(2), ALU.mult)                  # g2
        TT(S(24, 2), S(4, 2, step=15), S(5, 2, step=15), ALU.add)    # zs, cs
        TT(S(26, 2), S(4, 2, step=15), S(5, 2, step=15), ALU.mult)   # zp, cp
        TT(S(28, 2), S(4, 2, step=15), S(20, 2, step=-15), ALU.mult)  # zjck,cjzk
        TT(S(28), S(28), S(29), ALU.add)                 # zc
        TT(S(29), S(26), S(27), ALU.subtract)            # AA
        zic = S(3, 2, step=15)                           # (zi, c_i)
        TT(S2(0, 2), zic, S(24, 2), ALU.add)             # ch0, ch1
        TT(S2(6, 2), zic, S(24, 2), ALU.mult)            # zi*zs, ci*cs
        TT(S2(8, 2), zic, S(25, 2, step=-1), ALU.mult)   # zi*cs, ci*zs
        TT(S2(10, 2), zic, S(29, 2, step=-1), ALU.mult)  # zi*AA, ci*zc
        TT(S2(12, 2), zic, S(28, 2), ALU.mult)           # zi*zc, ci*AA
        TT(S2(2), S2(6), S2(7), ALU.subtract)
        TT(S2(2), S2(2), S(29), ALU.add)                 # ch2
        TT(S2(3), S2(8), S2(9), ALU.add)
        TT(S2(3), S2(3), S(28), ALU.add)                 # ch3
        TT(S2(4), S2(10), S2(11), ALU.subtract)          # ch4
        TT(S2(5), S2(12), S2(13), ALU.add)               # ch5
        # sum-of-squares for both norms -> slots 30 (geo), 31 (chem)
        TT(S(12, 3), S(21, 3), S(21, 3), ALU.mult)
        V.tensor_reduce(out=S(30), in_=slot_T(FB, 12, 3, CC),
                        axis=mybir.AxisListType.X, op=ALU.add)
        TT(S2(6, 6), S2(0, 6), S2(0, 6), ALU.mult)
        V.tensor_reduce(out=S(31), in_=slot_T(FB2, 6, 6, CC),
                        axis=mybir.AxisListType.X, op=ALU.add)
        # rsqrt of (gss, css) -> FB2 slots (14, 15)
        y = S2(14, 2)
        yu = slot(FB2, 14, 2, 1, CC).bitcast(U32)
        su = slot(FB, 30, 2, 1, CC).bitcast(U32)
        V.tensor_scalar(out=yu, in0=su, scalar1=1, scalar2=None,
                        op0=ALU.logical_shift_right)
        TT(yu, _ap(magic[:, 0:1], [[0, 2], [0, CC]]), yu, ALU.subtract)
        t = S(12, 2)
        for _ in range(2):
            TT(t, S(30, 2), y, ALU.mult)
            TT(t, t, y, ALU.mult)
            V.tensor_scalar(out=t, in0=t, scalar1=-0.5, scalar2=1.5,
                            op0=ALU.mult, op1=ALU.add)
            TT(y, y, t, ALU.mult)
        # scaled features -> fb9[:, :, 0:9]
        TT(_ap(fb9[:, 0, 0:1], [[1, 3], [32, CC]]), S(21, 3),
           slot_bc(FB2, 14, 3, CC), ALU.mult)
        TT(_ap(fb9[:, 0, 3:4], [[1, 6], [32, CC]]), S2(0, 6),
           slot_bc(FB2, 15, 6, CC), ALU.mult)
        # cutoff weights -> FB2 slot 10 (fci * fck)
        V.tensor_scalar(out=S2(8, 2), in0=S(0, 2), scalar1=CUTOFF,
                        scalar2=None, op0=ALU.min)
        nc.scalar.activation(S2(8, 2), S2(8, 2), ACTF.Sin,
                             bias=halfpi[:, 0:1], scale=-PI / CUTOFF)
        V.tensor_scalar(out=S2(8, 2), in0=S2(8, 2), scalar1=0.5, scalar2=0.5,
                        op0=ALU.mult, op1=ALU.add)
        TT(S2(10), S2(8), S2(9), ALU.mult)               # w

        if STAGE == 4:
            return early_exit(fb9[0:32, :, :], 32 * CC)

        # ---- transpose to feature-major, cast to fp16 ----
        V.transpose(out=fb9[:], in_=fb9[:])
        V.tensor_copy(xfm[:], fb9[:])
        xf = xfm[:].rearrange("p c u -> p (c u)")
        if STAGE == 3:
            return early_exit(fb9[0:32, :, :], 32 * CC)

        # ---- MLP (two strips per matmul via stacked/diag weights) ----
        def mm(ps, w_ap, rhs):
            nc.tensor.matmul(ps, w_ap, rhs, start=True, stop=True,
                             skip_group_check=True)

        def tanh(dst, src, bias):
            nc.scalar.activation(dst, src, ACTF.Tanh, bias=bias)

        ps0 = psp.tile([128, 2 * SL], F32, tag="ps", name="ps")
        mm(ps0[:, 0:SL], W("w0sA"), xf)
        mm(ps0[:, SL:2 * SL], W("w0sB"), xf)
        xres = actp.tile([128, 2 * SL], F16, tag="h", name="h")
        tanh(xres[:], ps0[:], B("b0"))
        ps1 = psp.tile([128, 2 * SL], F32, tag="ps", name="ps")
        mm(ps1[:], W("w1d"), xres[:])
        x1 = actp.tile([128, 2 * SL], F16, tag="h", name="h")
        tanh(x1[:], ps1[:], B("b1"))
        xb1 = actp.tile([128, 2 * SL], F16, tag="h", name="h")
        TT(xb1[:], x1[:], xres[:], ALU.add)
        cur = xb1
        for l in ("b2", "b3", "b4"):
            psl = psp.tile([128, 2 * SL], F32, tag="ps", name="ps")
            mm(psl[:], W("w%sd" % l[1]), cur[:])
            nxt = actp.tile([128, 2 * SL], F16, tag="h", name="h")
            tanh(nxt[:], psl[:], B(l))
            cur = nxt
        xb2 = actp.tile([128, 2 * SL], F16, tag="h", name="h")
        TT(xb2[:], cur[:], xb1[:], ALU.add)
        # xb3 is block-major: col = 128*c + 32*a + u for token (a, l=32c+u),
        # so each final-layer block's 128 tokens are one contiguous slice.
        def xdst(a):
            return _ap(xb3[:, 32 * a:32 * a + 1], [[128, CC], [1, 32]])

        ps5a = psp.tile([128, 2 * SL], F32, tag="ps", name="ps")
        mm(ps5a[:], W("w5p0"), xb2[:])
        tanh(xdst(0), ps5a[:, 0:SL], B("b5"))
        tanh(xdst(1), ps5a[:, SL:2 * SL], B("b5"))
        ps5b = psp.tile([128, 2 * SL], F32, tag="ps", name="ps")
        mm(ps5b[:], W("w5p1"), xb2[:])
        tanh(xdst(2), ps5b[:, 0:SL], B("b5"))
        tanh(xdst(3), ps5b[:, SL:2 * SL], B("b5"))

        if STAGE == 2:
            return early_exit(xb3[0:32, 0:256], 256)

        # ---- final layer + weighted scatter-add ----
        TT(ab[:], oh_v[:, ch, :, :],
           _ap(FB2[:, 10, :], [[1, CC], [0, 32]]), ALU.mult)
        for bb in range(CC):
            ps6 = ps6p.tile([128, 256], F32, tag="ps6", name="ps6")
            mm(ps6[:], xb3[:, 128 * bb:128 * bb + 128], W("w6", 256))
            t6 = smal.tile([128, 256], F16, tag="t6", name="t6")
            TT(t6[:], ps6[:], W("b6bc", 256), ALU.add)
            otm = smal.tile([128, 256], F16, tag="otm", name="otm")
            nc.scalar.activation(otm[:], t6[:], ACTF.Tanh)
            nc.tensor.matmul(ga[:], ab[:, bb, :], otm[:],
                             start=(ch == 0 and bb == 0),
                             stop=(ch == NCHUNK - 1 and bb == CC - 1),
                             skip_group_check=True)

    if STAGE == 1:
        return early_exit(ga[:], 256)

    # ---- normalize rows of ga, write out ----
    gac = consts.tile([32, 256], F32, tag="gac", name="gac")
    V.tensor_copy(gac[:], ga[:])
    if STAGE == 11:
        return early_exit(gac[:], 256)
    sq = consts.tile([32, 256], F32, tag="sq", name="sq")
    s = consts.tile([32, 4], F32, tag="s", name="s")
    V.tensor_tensor(out=sq[:], in0=gac[:], in1=gac[:], op=ALU.mult)
    V.tensor_reduce(out=s[:, 0:1], in_=sq[:], axis=mybir.AxisListType.X,
                    op=ALU.add)
    V.tensor_scalar(out=s[:, 0:1], in0=s[:, 0:1], scalar1=1e-20,
                    scalar2=None, op0=ALU.add)
    if STAGE == 12:
        return early_exit(s[:, 0:1], 1)
    yu = s[:, 1:2].bitcast(U32)
    V.tensor_scalar(out=yu, in0=s[:, 0:1].bitcast(U32), scalar1=1,
                    scalar2=None, op0=ALU.logical_shift_right)
    V.tensor_tensor(out=yu, in0=magic[0:32, 0:1], in1=yu, op=ALU.subtract)
    if STAGE == 13:
        return early_exit(s[:, 1:2], 1)
    for _ in range(2):
        V.tensor_tensor(out=s[:, 2:3], in0=s[:, 0:1], in1=s[:, 1:2],
                        op=ALU.mult)
        V.tensor_tensor(out=s[:, 2:3], in0=s[:, 2:3], in1=s[:, 1:2],
                        op=ALU.mult)
        V.tensor_scalar(out=s[:, 2:3], in0=s[:, 2:3], scalar1=-0.5,
                        scalar2=1.5, op0=ALU.mult, op1=ALU.add)
        V.tensor_tensor(out=s[:, 1:2], in0=s[:, 1:2], in1=s[:, 2:3],
                        op=ALU.mult)
    if STAGE == 14:
        return early_exit(s[:, 1:2], 1)
    # nrm = s * y  (= sqrt(s)); d = nrm + EPS; r = 1/d
    V.tensor_tensor(out=s[:, 2:3], in0=s[:, 0:1], in1=s[:, 1:2], op=ALU.mult)
    V.tensor_scalar(out=s[:, 2:3], in0=s[:, 2:3], scalar1=EPS, scalar2=None,
                    op0=ALU.add)
    V.reciprocal(out=s[:, 3:4], in_=s[:, 2:3])
    if STAGE == 15:
        return early_exit(s[:, 3:4], 1)
    outs = consts.tile([32, 256], F32, tag="outs", name="outs")
    V.tensor_scalar(out=outs[:], in0=gac[:], scalar1=s[:, 3:4], scalar2=None,
                    op0=ALU.mult)
    nc.sync.dma_start(out_ap[:], outs[:])


# --------------------------------------------------------------------------
# host-side: packing + input prep
# --------------------------------------------------------------------------

def _pack_atoms(cnt, T):
    """LPT bin-pack 256 atoms onto 8 cores (<=32 slots, <=T tokens).

    Returns per-core list of flat atom ids, or None if infeasible."""
    order = np.argsort(-cnt, kind="stable")
    loads = np.zeros(NCORE, np.int64)
    cores = [[] for _ in range(NCORE)]
    for a in order:
        cand = [c for c in range(NCORE) if len(cores[c]) < NA]
        c = min(cand, key=lambda c: loads[c])
        if loads[c] + cnt[a] > T:
            return None
        cores[c].append(int(a))
        loads[c] += cnt[a]
    return cores


def make_inputs(D, S, Ws, bs, CC, NCHUNK, cores, pairs):
    """Build per-core device input dicts."""
    T = 128 * CC * NCHUNK
    SL = 32 * CC

    # shared weight pack (fp16)
    wp = np.zeros((128, W16_COLS), np.float32)
    wp[0:9, 0:64] = Ws[0]
    wp[64:73, 64:128] = Ws[0]
    wp[32:41, 128:192] = Ws[0]
    wp[96:105, 192:256] = Ws[0]
    for i, l in enumerate((1, 2, 3, 4)):
        c = 256 + 128 * i
        wp[0:64, c:c + 64] = Ws[l]
        wp[64:128, c + 64:c + 128] = Ws[l]
    wp[0:64, 768:896] = Ws[5]
    wp[64:128, 896:1024] = Ws[5]
    wp[:, 1024:1280] = Ws[6]
    wp[:, 1280:1536] = np.broadcast_to(bs[6], (128, 256))
    wp16 = wp.astype(np.float16)

    wf32 = np.zeros((128, 6), np.float32)
    for l in range(5):
        wf32[:, l] = np.concatenate([bs[l], bs[l]])
    wf32[:, 5] = bs[5]

    ins = []
    for c in range(NCORE):
        raw = np.zeros((6, T), np.float32)
        raw[0:2, :] = 5.0
        raw[2:6, :] = 1.0
        slot_of = np.full((T,), -1, np.int64)
        t = 0
        for sidx, a in enumerate(cores[c]):
            b, i = a // NA, a % NA
            for p in pairs[a]:
                j, k = _JI[p], _KI[p]
                raw[0, t] = D[b, i, j]
                raw[1, t] = D[b, i, k]
                raw[2, t] = D[b, j, k]
                raw[3, t] = S[b, i]
                raw[4, t] = S[b, j]
                raw[5, t] = S[b, k]
                slot_of[t] = sidx
                t += 1
        # token t -> (chunk, strip, l): t = TC*ch + SL*a + l;  FB partition
        # row = 32a + l%32, col group = l//32
        tt = np.arange(T)
        ch = tt // (128 * CC)
        a_ = (tt % (128 * CC)) // SL
        l_ = tt % SL
        u = l_ % 32
        cg = l_ // 32
        geom = np.zeros((128, NCHUNK, 6, CC), np.float32)
        geom[32 * a_ + u, ch, :, cg] = raw.T
        oh = np.zeros((128, NCHUNK, CC, 32), np.float32)
        real = slot_of >= 0
        oh[32 * a_[real] + u[real], ch[real], cg[real], slot_of[real]] = 1.0
        ins.append({
            "geom": np.ascontiguousarray(geom.reshape(128, -1)),
            "oh": oh.reshape(128, -1).astype(ml_dtypes.bfloat16),
            "wpack16": wp16,
            "wf32": wf32,
        })
    return ins


# --------------------------------------------------------------------------
# module build + run
# --------------------------------------------------------------------------

_BUILT = {}


def build_bass(CC, NCHUNK):
    key = (CC, NCHUNK)
    if key in _BUILT:
        return _BUILT[key]
    nc = bacc.Bacc(trn_type="TRN2", target_bir_lowering=False, debug=False)
    ins = {
        "geom": nc.dram_tensor("geom", [128, NCHUNK * 6 * CC], F32,
                               kind="ExternalInput").ap(),
        "oh": nc.dram_tensor("oh", [128, NCHUNK * CC * 32], BF16,
                             kind="ExternalInput").ap(),
        "wpack16": nc.dram_tensor("wpack16", [128, W16_COLS], F16,
                                  kind="ExternalInput").ap(),
        "wf32": nc.dram_tensor("wf32", [128, 6], F32,
                               kind="ExternalInput").ap(),
    }
    out = nc.dram_tensor("out", [NA, 256], F32, kind="ExternalOutput").ap()
    with tile.TileContext(nc) as tc:
        with ExitStack() as ctx:
            build_kernel(ctx, tc, out, ins, CC, NCHUNK)
    nc.finalize()
    _BUILT[key] = nc
    return nc


def _run(inputs, **spmd_kwargs):
    D = np.asarray(inputs["distance_matrices"], np.float32)
    S = np.asarray(inputs["num_species_batch"], np.float32)
    Ws = [np.asarray(inputs[f"W{i}"], np.float32) for i in range(7)]
    bs = [np.asarray(inputs[f"b{i}"], np.float32) for i in range(7)]

    # surviving triplets per (molecule, atom)
    cm = (D < CUTOFF) & (D != 0.0)
    m = cm[:, :, _JI] & cm[:, :, _KI]             # [8, 32, 496]
    cnt = m.sum(axis=2).reshape(-1)
    pairs = [np.nonzero(m.reshape(-1, len(_JI))[a])[0] for a in range(8 * NA)]

    for CC, NCHUNK in LADDER:
        T = 128 * CC * NCHUNK
        cores = _pack_atoms(cnt, T)
        if cores is not None:
            break
    else:
        CC, NCHUNK = LADDER[-1]
        cores = [[b * NA + i for i in range(NA)] for b in range(NCORE)]

    nc = build_bass(CC, NCHUNK)
    in_maps = make_inputs(D, S, Ws, bs, CC, NCHUNK, cores, pairs)
    res = run_bass_kernel_spmd(nc, in_maps, core_ids=list(range(NCORE)),
                               **spmd_kwargs)
    out = np.zeros((NCORE, NA, 256), np.float32)
    for c in range(NCORE):
        rc = np.asarray(res.results[c]["out"], np.float32)
        for sidx, a in enumerate(cores[c]):
            out[a // NA, a % NA] = rc[sidx]
    return out, res


def kernel(**inputs):
    out, _ = _run(inputs)
    return out
